# revision 1
# baseline (speedup 1.0000x reference)
"""ConvAttention TRN2 kernel: depthwise-conv QKV + full softmax attention + projection.

Self-contained: hardcodes shapes B=2, C=96, H=W=64, N=4096, heads=3, d=32.
Shards query rows across 8 NeuronCores (512 rows each); k/v conv replicated
on-device; q conv computed from a per-core halo slice of the input.
"""

import os
import sys

import numpy as np

for _p in ("/opt/trn_rl_repo", "/root/.axon_site/_ro/trn_rl_repo"):
    if os.path.isdir(_p) and _p not in sys.path:
        sys.path.append(_p)

from contextlib import ExitStack

import concourse.bass as bass
import concourse.masks as masks
import concourse.tile as tile
from concourse import bacc, mybir
from concourse.bass_utils import run_bass_kernel_spmd

F32 = mybir.dt.float32
F32R = mybir.dt.float32r
BF16 = mybir.dt.bfloat16

B = 2
C = 96
H = W = 64
N = H * W            # 4096
NHEADS = 3
D = C // NHEADS      # 32
SCALE = float(D) ** -0.5
NCORES = 8
NQ = N // NCORES     # 512 query rows per core
QROWS = NQ // W      # 8 spatial rows per core
WP = W + 2           # padded width 66
NKCH = N // 128      # 32 key chunks of 128


def _build_program(debug_outputs=False):
    nc = bacc.Bacc("TRN2", target_bir_lowering=False, debug=False, num_devices=NCORES)

    xq_d = nc.dram_tensor("xq", [B, 32, QROWS + 4, WP], BF16, kind="ExternalInput").ap()
    xkv_d = nc.dram_tensor("xkv", [B, 64, H + 4, WP], BF16, kind="ExternalInput").ap()
    wm_d = nc.dram_tensor("wm", [3, 3, 97, 96], BF16, kind="ExternalInput").ap()
    pw_d = nc.dram_tensor("pw", [96, 96], BF16, kind="ExternalInput").ap()
    pb_d = nc.dram_tensor("pb", [96, 1], F32, kind="ExternalInput").ap()
    y_d = nc.dram_tensor("y", [B, N, 96], F32, kind="ExternalOutput").ap()
    # per-(b,h) collective staging: ah block [32, 512] -> allgather -> flat scrambled layout
    stg_d = [nc.dram_tensor(f"stg{b}", [96, NQ], F32).ap() for b in range(B)]
    gth_d = [[nc.dram_tensor(f"gth{b}_{h}", [NCORES, 32, NQ], F32,
                             addr_space="Shared").ap()
              for h in range(NHEADS)] for b in range(B)]
    flt_d = [nc.dram_tensor(f"flt{b}", [96, NCORES, NQ], F32).ap() for b in range(B)]
    dbg = {}
    if debug_outputs:
        dbg["q"] = nc.dram_tensor("dbg_q", [96, B, NQ], BF16, kind="ExternalOutput").ap()
        dbg["k"] = nc.dram_tensor("dbg_k", [96, B, N], BF16, kind="ExternalOutput").ap()
        dbg["vt"] = nc.dram_tensor("dbg_vt", [128, B, NHEADS, NKCH, 33], BF16, kind="ExternalOutput").ap()
        dbg["acc"] = nc.dram_tensor("dbg_acc", [B, NHEADS, 33, 512], F32, kind="ExternalOutput").ap()
        dbg["ah"] = nc.dram_tensor("dbg_ah", [B, NHEADS, 32, 512], F32, kind="ExternalOutput").ap()

    LQ = QROWS * WP          # 528 usable elems per (dy) shift for q
    LK = (H + 2) * WP        # 4356 for k/v

    with tile.TileContext(nc) as tc, ExitStack() as ctx:
        consts = ctx.enter_context(tc.tile_pool(name="consts", bufs=1))
        xrep_p = ctx.enter_context(tc.tile_pool(name="xrep", bufs=1))
        qkv_p = ctx.enter_context(tc.tile_pool(name="qkv", bufs=1))
        vtmp_p = ctx.enter_context(tc.tile_pool(name="vtmp", bufs=2))
        vt_p = ctx.enter_context(tc.tile_pool(name="vt", bufs=1))
        exp_p = ctx.enter_context(tc.tile_pool(name="exp", bufs=18))
        arhs_p = ctx.enter_context(tc.tile_pool(name="arhs", bufs=2))
        small_p = ctx.enter_context(tc.tile_pool(name="small", bufs=2))
        out_p = ctx.enter_context(tc.tile_pool(name="out", bufs=2))

        conv_ps = ctx.enter_context(tc.tile_pool(name="conv_ps", bufs=2, space="PSUM"))
        acc_ps_p = conv_ps
        sc_ps_p = ctx.enter_context(tc.tile_pool(name="sc_ps", bufs=2, space="PSUM"))
        misc_ps_p = ctx.enter_context(tc.tile_pool(name="misc_ps", bufs=2, space="PSUM"))

        # ---- constants ----
        wm_sb = consts.tile([97, 9, 96], BF16)
        for g in range(3):
            for dx in range(3):
                nc.sync.dma_start(wm_sb[:, g * 3 + dx, :], wm_d[g, dx, :, :])
        pw_sb = consts.tile([96, 96], BF16)
        nc.sync.dma_start(pw_sb[:], pw_d[:, :])
        pb_sb = consts.tile([96, 1], F32)
        nc.sync.dma_start(pb_sb[:], pb_d[:, :])
        ident = consts.tile([128, 128], BF16)
        masks.make_identity(nc, ident[:])
        identf = consts.tile([128, 128], F32)
        masks.make_identity(nc, identf[:])
        ones_col = consts.tile([1, 32], F32)
        nc.vector.memset(ones_col[:], 1.0)

        # ---- x replicated-shift loads: partition p = dy*32 + c holds channel c shifted dy rows ----
        xrep_q = xrep_p.tile([97, B, LQ], BF16)
        xrep_k = xrep_p.tile([97, B, LK], BF16)
        xrep_v = xrep_p.tile([97, B, LK], BF16)
        xq_flat = xq_d[:, :, :, :].rearrange("b c r w -> c b (r w)")
        xkv_flat = xkv_d[:, :, :, :].rearrange("b c r w -> c b (r w)")
        for dy in range(3):
            nc.sync.dma_start(
                xrep_q[dy * 32:(dy + 1) * 32, :, :],
                xq_flat[:, :, dy * WP: dy * WP + LQ])
            for b in range(B):
                nc.sync.dma_start(
                    xrep_k[dy * 32:(dy + 1) * 32, b, :],
                    xkv_flat[0:32, b, dy * WP: dy * WP + LK])
                nc.gpsimd.dma_start(
                    xrep_v[dy * 32:(dy + 1) * 32, b, :],
                    xkv_flat[32:64, b, dy * WP: dy * WP + LK])
        nc.vector.memset(xrep_q[96:97, :, :], 1.0)
        nc.vector.memset(xrep_k[96:97, :, :], 1.0)
        nc.vector.memset(xrep_v[96:97, :, :], 1.0)

        # ---- qkv persistent sbuf ----
        q_all = qkv_p.tile([96, B, NQ], BF16)     # (h*32+d, b, nq)
        k_all = qkv_p.tile([96, B, N], BF16)      # (h*32+d, b, nk)
        vt_aug = vt_p.tile([128, B, NHEADS, NKCH, 33], BF16)  # (nk%128, b, h, chunk, d|1)
        nc.vector.memset(vt_aug[:, :, :, :, 32:33], 1.0)

        # ---- depthwise conv as matmuls: out[o, n] = sum_{dy,c} W[dy*32+c, o] * xrep[(dy,c), n+dx-shift] ----
        def conv_group(g, xr, nblocks, rows_per_blk, emit_block):
            view = [None] * B
            for b in range(B):
                view[b] = xr[:, b, :].rearrange("k (r w) -> k r w", w=WP)
            for b in range(B):
                for blk in range(nblocks):
                    ps = conv_ps.tile([96, 512], F32, tag="pacc")
                    for dx in range(3):
                        rhs = view[b][:, blk * rows_per_blk: blk * rows_per_blk + rows_per_blk, dx: dx + W]
                        nc.tensor.matmul(
                            ps[:, :],
                            lhsT=wm_sb[:, g * 3 + dx, :],
                            rhs=rhs,
                            start=(dx == 0), stop=(dx == 2))
                    emit_block(b, blk, ps)

        # q: one 512-block per b
        conv_group(0, xrep_q, 1, QROWS,
                   lambda b, blk, ps: nc.scalar.copy(q_all[:, b, :], ps[:, :]))
        # k: 8 blocks per b
        conv_group(1, xrep_k, 8, QROWS,
                   lambda b, blk, ps: nc.scalar.copy(k_all[:, b, blk * 512:(blk + 1) * 512], ps[:, :]))

        # v: 8 blocks per b -> transpose into vt_aug
        def emit_v(b, blk, ps):
            vtmp = vtmp_p.tile([96, 512], BF16)
            nc.scalar.copy(vtmp[:, :], ps[:, :])
            for c4 in range(4):
                ch = blk * 4 + c4
                tp = misc_ps_p.tile([128, 96], BF16, tag="m")
                nc.tensor.transpose(tp[:, :], vtmp[:, c4 * 128:(c4 + 1) * 128], ident[0:96, 0:96])
                nc.vector.tensor_copy(
                    vt_aug[:, b, :, ch, 0:32],
                    tp[:, :].rearrange("p (h d) -> p h d", d=32))
        conv_group(2, xrep_v, 8, QROWS, emit_v)

        # ---- attention + gather (both batches) ----
        for b in range(B):
            ah_tiles = []
            for h in range(NHEADS):
                # stage 1: stream all score matmuls (pairs share a 2-bank psum
                # tile so each exp covers 1024 columns)
                exs = []
                for cp in range(NKCH // 2):
                    sc2 = sc_ps_p.tile([128, 1024], F32)
                    for half in range(2):
                        ch = 2 * cp + half
                        nc.tensor.matmul(
                            sc2[:, half * 512:(half + 1) * 512],
                            lhsT=k_all[32 * h:32 * (h + 1), b, ch * 128:(ch + 1) * 128],
                            rhs=q_all[32 * h:32 * (h + 1), b, :],
                            start=True, stop=True)
                    ex2 = exp_p.tile([128, 1024], BF16)
                    nc.scalar.activation(ex2[:, :], sc2[:, :],
                                         mybir.ActivationFunctionType.Exp, scale=SCALE)
                    exs.append(ex2)
                # stage 2: accumulate PV back-to-back
                acc = acc_ps_p.tile([33, 512], F32, tag="pacc")
                for cp in range(NKCH // 2):
                    for half in range(2):
                        ch = 2 * cp + half
                        nc.tensor.matmul(
                            acc[:, :],
                            lhsT=vt_aug[:, b, h, ch, :],
                            rhs=exs[cp][:, half * 512:(half + 1) * 512],
                            start=(ch == 0), stop=(ch == NKCH - 1))
                # normalize: rows 0:32 are sum(exp * v), row 32 is sum(exp)
                rden = small_p.tile([1, 512], F32)
                nc.vector.reciprocal(rden[:, :], acc[32:33, :])
                bc = misc_ps_p.tile([32, 512], F32, tag="m")
                nc.tensor.matmul(bc[:, :], lhsT=ones_col[:, :],
                                 rhs=rden[:, :], start=True, stop=True)
                num = small_p.tile([32, 512], F32, tag="num")
                nc.vector.tensor_copy(num[:, :], acc[0:32, :])
                ah = arhs_p.tile([32, 512], F32, tag=f"a{h}")
                nc.vector.tensor_mul(ah[:, :], num[:, :], bc[:, :])
                ah_tiles.append(ah)
                nc.gpsimd.dma_start(stg_d[b][32 * h:32 * (h + 1), :], ah[:, :])
                if debug_outputs:
                    accs = small_p.tile([33, 512], F32, tag="dbgacc")
                    nc.vector.tensor_copy(accs[:, :], acc[:, :])
                    nc.sync.dma_start(dbg["acc"][b, h, :, :], accs[:, :])
                    nc.sync.dma_start(dbg["ah"][b, h, :, :], ah[:, :])
                # gather this head's token blocks from all cores, rebuild the
                # channel-major flat layout (reference reshape(B,N,C) flattens
                # (h,d,N) row-major)
                nc.gpsimd.collective_compute(
                    "AllGather", mybir.AluOpType.bypass,
                    ins=[stg_d[b][32 * h:32 * (h + 1), :]],
                    outs=[gth_d[b][h][:, :, :]],
                    replica_groups=[list(range(NCORES))])
                nc.sync.dma_start(
                    flt_d[b][32 * h:32 * (h + 1), :, :],
                    gth_d[b][h][:, :, :].rearrange("j c n -> c j n"))

        # ---- projection (both batches), after all collectives are in flight ----
        for b in range(B):
            # scrambled projection input: out2[n', c'] = flat[n' * 96 + c']
            out2 = flt_d[b][:, :, :].rearrange("c j n -> (c j n)").rearrange(
                "(n c) -> n c", c=96)
            yin_all = out_p.tile([128, NKCH, 96], F32, tag="yin")
            nc.sync.dma_start(yin_all[:, :, :],
                              out2.rearrange("(ch p) c -> p ch c", p=128))
            yo_all = out_p.tile([128, NKCH, 96], F32, tag="yo")
            for cg in range(NCORES):
                rhs = small_p.tile([96, 512], BF16, tag="prhs")
                for c4 in range(4):
                    chk = cg * 4 + c4
                    tpi = misc_ps_p.tile([96, 128], F32, tag="m")
                    nc.tensor.transpose(tpi[:, :], yin_all[:, chk, :], identf[:, :])
                    nc.vector.tensor_copy(rhs[:, c4 * 128:(c4 + 1) * 128], tpi[:, :])
                yps = misc_ps_p.tile([96, 512], F32, tag="m")
                nc.tensor.matmul(yps[:, :], lhsT=pw_sb[:, :], rhs=rhs[:, :],
                                 start=True, stop=True)
                ysb = small_p.tile([96, 512], BF16, tag="ysb")
                nc.vector.tensor_scalar_add(ysb[:, :], yps[:, :], pb_sb[:, :])
                for c4 in range(4):
                    chk = cg * 4 + c4
                    tp = misc_ps_p.tile([128, 96], BF16, tag="m")
                    nc.tensor.transpose(tp[:, :], ysb[:, c4 * 128:(c4 + 1) * 128], ident[0:96, 0:96])
                    nc.vector.tensor_copy(yo_all[:, chk, :], tp[:, :])
            nc.sync.dma_start(
                y_d[b].rearrange("(ch p) c -> p ch c", p=128), yo_all[:, :, :])

        if debug_outputs:
            nc.sync.dma_start(dbg["q"][:, :, :], q_all[:, :, :])
            nc.sync.dma_start(dbg["k"][:, :, :], k_all[:, :, :])
            nc.sync.dma_start(dbg["vt"][:, :, :, :, :], vt_aug[:, :, :, :, :])

    nc.compile()
    return nc


_PROG = None


def _prep_inputs(x, qkv_w, qkv_b, proj_w, proj_b):
    x = np.asarray(x, np.float32)
    qkv_w = np.asarray(qkv_w, np.float32)
    qkv_b = np.asarray(qkv_b, np.float32)
    proj_w = np.asarray(proj_w, np.float32)
    proj_b = np.asarray(proj_b, np.float32)

    xt = x.transpose(0, 2, 1).reshape(B, C, H, W)
    xpad = np.zeros((B, C, H + 2, WP), np.float32)
    xpad[:, :, 1:H + 1, 1:W + 1] = xt

    xkv = np.zeros((B, 64, H + 4, WP), np.float32)
    xkv[:, :, 0:H + 2, :] = xpad[:, 32:96]

    xqs = []
    for i in range(NCORES):
        buf = np.zeros((B, 32, QROWS + 4, WP), np.float32)
        buf[:, :, 0:QROWS + 2, :] = xpad[:, 0:32, i * QROWS: i * QROWS + QROWS + 2, :]
        xqs.append(buf)

    w = qkv_w.reshape(3 * C, 3, 3)
    wm = np.zeros((3, 3, 97, 96), np.float32)  # [g, dx, k=(dy*32+c), o]
    o = np.arange(96)
    for g in range(3):
        for dy in range(3):
            for dx in range(3):
                wm[g, dx, dy * 32 + o // 3, o] = w[g * 96 + o, dy, dx]
        wm[g, 0, 96, :] = qkv_b[g * 96:(g + 1) * 96]

    import ml_dtypes
    bf16 = ml_dtypes.bfloat16
    xqs = [a.astype(bf16) for a in xqs]
    xkv = xkv.astype(bf16)
    wm = wm.astype(bf16)
    pw = np.ascontiguousarray(proj_w.T).astype(bf16)
    pb = np.ascontiguousarray(proj_b.reshape(96, 1))
    return xqs, xkv, wm, pw, pb


def kernel(x, qkv_w, qkv_b, proj_w, proj_b, H=64, W=64):
    global _PROG
    if _PROG is None:
        _PROG = _build_program()
    nc = _PROG

    xqs, xkv, wm, pw, pb = _prep_inputs(x, qkv_w, qkv_b, proj_w, proj_b)
    in_maps = [
        {"xq": xqs[i], "xkv": xkv, "wm": wm, "pw": pw, "pb": pb}
        for i in range(NCORES)
    ]
    res = run_bass_kernel_spmd(nc, in_maps, list(range(NCORES)))
    return np.asarray(res.results[0]["y"])



# revision 12
# speedup vs baseline: 1.4023x; 1.4023x over previous
"""ConvAttention TRN2 kernel: depthwise-conv QKV + full softmax attention + projection.

Self-contained: hardcodes shapes B=2, C=96, H=W=64, N=4096, heads=3, d=32.

Sharding: each of the 8 cores computes attention for its own 512 query tokens
(q conv from a halo slice; k/v conv over the full grid redundantly per core).
The reference's reshape(B, N, C) is a scrambled reshape of [B, h, d, N], so the
projection input row n is the 96-wide window attn.flat[96n : 96n+96].  Windows
start at 32-token boundaries; each core projects the windows starting in its
token range, using a 96-token right-halo / 64-token left-halo of neighbor
attention output obtained via one AllGather (hidden under the second batch's
key loop).  Window extraction is done with one-hot selection matmuls whose
matrices are per-core *input data*, keeping the SPMD program core-uniform.

Schedule: ACT does nothing but the 96 exp() tiles (the hard floor ~99us); PE
interleaves conv / PV / projection between score matmuls so it stays busy
(holding the 2.4GHz p-state) slightly ahead of ACT.
"""

import os
import sys

import numpy as np

for _p in ("/opt/trn_rl_repo", "/root/.axon_site/_ro/trn_rl_repo"):
    if os.path.isdir(_p) and _p not in sys.path:
        sys.path.append(_p)

from collections import deque
from contextlib import ExitStack

import concourse.bass as bass
import concourse.tile as tile
from concourse import bacc, mybir
from concourse.bass_utils import run_bass_kernel_spmd

F32 = mybir.dt.float32
F32R = mybir.dt.float32r
BF16 = mybir.dt.bfloat16

B = 2
C = 96
H = W = 64
N = H * W            # 4096
NHEADS = 3
D = C // NHEADS      # 32
SCALE = float(D) ** -0.5
NCORES = 8
NQ = 512             # query rows per core
QROWS = NQ // W      # 8 spatial rows per core
WP = W + 2           # padded width 66
NKCH = N // 128      # 32 key chunks of 128
LQ = QROWS * WP      # 528 usable elems per (dy) shift for q
LK = (H + 2) * WP    # 4356 for k/v
KEARLY = 10 * WP     # first 10 rows per dy plane (covers conv blocks 0-1)
EXT = 64 + NQ + 96   # ah_ext tokens: left halo | own | right halo = 672


def _build_program():
    nc = bacc.Bacc("TRN2", target_bir_lowering=False, debug=False, num_devices=NCORES)

    xq_d = nc.dram_tensor("xq", [B, 32, QROWS + 4, WP], BF16, kind="ExternalInput").ap()
    xkv_d = nc.dram_tensor("xkv", [B, 64, H + 4, WP], BF16, kind="ExternalInput").ap()
    wm_d = nc.dram_tensor("wm", [3, 3, 97, 96], BF16, kind="ExternalInput").ap()
    pw_d = nc.dram_tensor("pw", [96, 96], BF16, kind="ExternalInput").ap()
    pb_d = nc.dram_tensor("pb", [96, 1], F32, kind="ExternalInput").ap()
    ones_d = nc.dram_tensor("ones", [1, B * LK], BF16, kind="ExternalInput").ap()
    s3_d = nc.dram_tensor("s3", [3, 96, 32], BF16, kind="ExternalInput").ap()
    sp2_d = nc.dram_tensor("sp2", [2, 96, 32], BF16, kind="ExternalInput").ap()
    selr_d = nc.dram_tensor("selr", [96, 8], F32, kind="ExternalInput").ap()
    sell_d = nc.dram_tensor("sell", [96, 8], F32, kind="ExternalInput").ap()
    y_d = nc.dram_tensor("y", [B, 96, NQ + 64], F32, kind="ExternalOutput").ap()
    stg_d = nc.dram_tensor("stg", [96, B, 160], BF16).ap()
    gth_d = nc.dram_tensor("gth", [NCORES, 96, B, 160], BF16, addr_space="Shared").ap()

    with tile.TileContext(nc) as tc, ExitStack() as ctx:
        consts = ctx.enter_context(tc.tile_pool(name="consts", bufs=1))
        xrep_p = ctx.enter_context(tc.tile_pool(name="xrep", bufs=1))
        qkv_p = ctx.enter_context(tc.tile_pool(name="qkv", bufs=1))
        exp_p = ctx.enter_context(tc.tile_pool(name="exp", bufs=4))
        small_p = ctx.enter_context(tc.tile_pool(name="small", bufs=2))

        sc_ps = ctx.enter_context(tc.tile_pool(name="sc_ps", bufs=2, space="PSUM"))
        acc_ps = ctx.enter_context(tc.tile_pool(name="acc_ps", bufs=2, space="PSUM"))
        misc_ps = ctx.enter_context(tc.tile_pool(name="misc_ps", bufs=2, space="PSUM"))

        # ---- constants ----
        wm_sb = consts.tile([97, 9, 96], BF16)
        nc.sync.dma_start(wm_sb[:, :, :],
                          wm_d[:, :, :, :].rearrange("g dx k o -> k (g dx) o"))
        pw_sb = consts.tile([96, 96], BF16)
        nc.scalar.dma_start(pw_sb[:], pw_d[:, :])
        pb_sb = consts.tile([96, 1], F32)
        nc.scalar.dma_start(pb_sb[:], pb_d[:, :])
        s3_sb = consts.tile([96, 3, 32], BF16)
        nc.scalar.dma_start(s3_sb[:, :, :], s3_d[:, :, :].rearrange("a c s -> c a s"))
        sp2_sb = consts.tile([96, 2, 32], BF16)
        nc.scalar.dma_start(sp2_sb[:, :, :], sp2_d[:, :, :].rearrange("a c s -> c a s"))
        selr_sb = consts.tile([96, 8], F32)
        nc.scalar.dma_start(selr_sb[:], selr_d[:, :])
        sell_sb = consts.tile([96, 8], F32)
        nc.scalar.dma_start(sell_sb[:], sell_d[:, :])
        ones_col = consts.tile([1, 32], BF16)
        nc.vector.memset(ones_col[:], 1.0)

        # ---- x replicated-shift loads: partition p = dy*32 + c holds channel c shifted dy rows ----
        xrep_q = xrep_p.tile([97, B, LQ], BF16)
        xrep_k = xrep_p.tile([97, B, LK], BF16)
        xrep_v = xrep_p.tile([97, B, LK], BF16)
        xq_flat = xq_d[:, :, :, :].rearrange("b c r w -> c b (r w)")
        xkv_flat = xkv_d[:, :, :, :].rearrange("b c r w -> c b (r w)")
        for dy in range(3):
            nc.sync.dma_start(
                xrep_q[dy * 32:(dy + 1) * 32, :, :],
                xq_flat[:, :, dy * WP: dy * WP + LQ])
        nc.sync.dma_start(xrep_q[96:97, :, :].rearrange("p b l -> p (b l)"),
                          ones_d[:, 0:B * LQ])
        nc.sync.dma_start(xrep_k[96:97, :, :].rearrange("p b l -> p (b l)"),
                          ones_d[:, :])
        nc.sync.dma_start(xrep_v[96:97, :, :].rearrange("p b l -> p (b l)"),
                          ones_d[:, :])
        # k on the Pool queue: first 10 rows of each dy plane first (conv blocks 0-1)
        for dy in range(3):
            nc.gpsimd.dma_start(
                xrep_k[dy * 32:(dy + 1) * 32, :, 0:KEARLY],
                xkv_flat[0:32, :, dy * WP: dy * WP + KEARLY])
        for dy in range(3):
            nc.gpsimd.dma_start(
                xrep_k[dy * 32:(dy + 1) * 32, :, KEARLY:LK],
                xkv_flat[0:32, :, dy * WP + KEARLY: dy * WP + LK])
        # v on the ACT queue (idle until the first exp)
        for dy in range(3):
            nc.scalar.dma_start(
                xrep_v[dy * 32:(dy + 1) * 32, :, :],
                xkv_flat[32:64, :, dy * WP: dy * WP + LK])

        # ---- persistent tiles ----
        q_all = qkv_p.tile([96, B, NQ], BF16)           # (h*32+d, b, nq)
        k_all = qkv_p.tile([96, B, N], BF16)            # (h*32+d, b, nk)
        vt_all = qkv_p.tile([128, B, NHEADS, NKCH, 33], BF16)  # (nk%128, b, h, chunk, d|1)
        ah_ext = qkv_p.tile([96, B, EXT], BF16)         # left64 | own512 | right96
        g_sb = qkv_p.tile([96, NCORES, B, 160], BF16)   # gathered halo slabs
        xt_sb = qkv_p.tile([96, B, NQ + 64], BF16)      # scrambled proj input
        ysb = qkv_p.tile([96, B, NQ + 64], F32)
        nc.vector.memset(vt_all[:, :, :, :, 32:33], 1.0)

        # warm the ACT exp table before the pipeline needs it
        dummy = small_p.tile([1, 32], BF16, tag="dummy")
        nc.scalar.activation(dummy[:, :], ones_col[:, :],
                             mybir.ActivationFunctionType.Exp, scale=SCALE)

        xq_view = [xrep_q[:, b, :].rearrange("k (r w) -> k r w", w=WP) for b in range(B)]
        xk_view = [xrep_k[:, b, :].rearrange("k (r w) -> k r w", w=WP) for b in range(B)]
        xv_view = [xrep_v[:, b, :].rearrange("k (r w) -> k r w", w=WP) for b in range(B)]

        # ---- PE filler work units ----
        def conv_q(b):
            ps = misc_ps.tile([128, 512], F32, tag="m")
            for dx in range(3):
                nc.tensor.matmul(
                    ps[0:96, :], lhsT=wm_sb[:, dx, :],
                    rhs=xq_view[b][:, 0:QROWS, dx: dx + W],
                    start=(dx == 0), stop=(dx == 2))
            nc.vector.tensor_copy(q_all[:, b, :], ps[0:96, :])

        def conv_k(b, blk):
            ps = misc_ps.tile([128, 512], F32, tag="m")
            for dx in range(3):
                nc.tensor.matmul(
                    ps[0:96, :], lhsT=wm_sb[:, 3 + dx, :],
                    rhs=xk_view[b][:, blk * QROWS: blk * QROWS + QROWS, dx: dx + W],
                    start=(dx == 0), stop=(dx == 2))
            nc.vector.tensor_copy(k_all[:, b, blk * 512:(blk + 1) * 512], ps[0:96, :])

        def conv_v(b, ch):
            # transposed orientation: stationary = one-row x window (the BIR
            # verifier requires a single free dim on the stationary operand)
            ps = misc_ps.tile([128, 512], F32, tag="m")
            for rr in range(2):
                row = 2 * ch + rr
                for dx in range(3):
                    nc.tensor.matmul(
                        ps[64 * rr:64 * rr + 64, 0:96],
                        lhsT=xv_view[b][:, row, dx: dx + W],
                        rhs=wm_sb[:, 6 + dx, :],
                        start=(dx == 0), stop=(dx == 2))
            nc.vector.tensor_copy(
                vt_all[:, b, :, ch, 0:32],
                ps[:, 0:96].rearrange("p (h d) -> p h d", d=32))

        # filler schedule: unit -> group -> [closures]
        fill = {}

        def add_fill(unit, grp, fn):
            fill.setdefault((unit, grp), []).append(fn)

        for j in range(2, 8):
            add_fill(0, 2 * (j - 2), lambda b=0, j=j: conv_k(b, j))
        for ch in range(4, 32):
            add_fill(0, (ch - 4) // 2, lambda b=0, ch=ch: conv_v(b, ch))
        for j in range(8):
            add_fill(1, 2 * j, lambda b=1, j=j: conv_k(b, j))
        add_fill(2, 0, lambda: conv_q(1))
        for ch in range(16):
            add_fill(2, ch, lambda b=1, ch=ch: conv_v(b, ch))
        for ch in range(16, 32):
            add_fill(3, (ch - 16) // 2, lambda b=1, ch=ch: conv_v(b, ch))

        # ---- prologue: minimum conv for unit 0's first groups ----
        conv_q(0)
        conv_k(0, 0)
        conv_k(0, 1)
        for ch in range(4):
            conv_v(0, ch)

        # ---- attention pair-segments ----
        # qseg: list of (q_offset, width); chunks per exp group sized so each
        # exp tile is [128, 1024].
        def pair_seg(unit, b, h, qseg):
            qw = sum(w for _, w in qseg)
            chunks_per_grp = 1024 // (2 * qw) * 2  # 2 for qw=512, 4 for qw=256
            ngrp = NKCH // chunks_per_grp
            acc = acc_ps.tile([33, 512], F32, tag="acc")
            pend = deque()

            def pv(g, ex):
                for ci in range(chunks_per_grp):
                    ch = chunks_per_grp * g + ci
                    nc.tensor.matmul(
                        acc[:, 0:qw], lhsT=vt_all[:, b, h, ch, :],
                        rhs=ex[:, ci * qw:(ci + 1) * qw],
                        start=(ch == 0), stop=(ch == NKCH - 1),
                        skip_group_check=True)

            for g in range(ngrp):
                sc = sc_ps.tile([128, 1024], F32, tag="sc")
                for ci in range(chunks_per_grp):
                    ch = chunks_per_grp * g + ci
                    col = ci * qw
                    for qo, qn in qseg:
                        nc.tensor.matmul(
                            sc[:, col:col + qn],
                            lhsT=k_all[32 * h:32 * (h + 1), b, ch * 128:(ch + 1) * 128],
                            rhs=q_all[32 * h:32 * (h + 1), b, qo:qo + qn],
                            start=True, stop=True)
                        col += qn
                ex = exp_p.tile([128, 1024], BF16)
                nc.scalar.activation(ex[:, :], sc[:, :],
                                     mybir.ActivationFunctionType.Exp, scale=SCALE)
                pend.append((g, ex))
                for fn in fill.get((unit, g), ()):
                    fn()
                if len(pend) > 2:
                    pv(*pend.popleft())
            while pend:
                pv(*pend.popleft())
            # normalize: rows 0:32 are sum(exp*v), row 32 is sum(exp)
            rden = small_p.tile([1, 512], BF16, tag="den")
            with nc.allow_low_precision(reason="softmax denom reciprocal in bf16"):
                nc.vector.reciprocal(rden[:, 0:qw], acc[32:33, 0:qw])
            bcp = misc_ps.tile([128, 512], F32, tag="m")
            nc.tensor.matmul(bcp[0:32, 0:qw], lhsT=ones_col[:, :], rhs=rden[:, 0:qw],
                             start=True, stop=True)
            num = small_p.tile([32, 512], BF16, tag="num")
            nc.vector.tensor_copy(num[:, 0:qw], acc[0:32, 0:qw])
            col = 0
            for qo, qn in qseg:
                nc.vector.tensor_mul(
                    ah_ext[32 * h:32 * (h + 1), b, 64 + qo: 64 + qo + qn],
                    num[:, col:col + qn], bcp[0:32, col:col + qn])
                col += qn

        FULL = [(0, 512)]
        SEGA = [(0, 128), (384, 128)]
        SEGB = [(128, 256)]

        for h in range(NHEADS):
            pair_seg(h, 0, h, FULL)
        for h in range(NHEADS):
            pair_seg(3 + h, 1, h, SEGA)

        # ---- stage halo slabs + AllGather (hidden under segB) ----
        nc.sync.dma_start(stg_d[:, :, 0:96], ah_ext[:, :, 64:160])
        nc.sync.dma_start(stg_d[:, :, 96:160], ah_ext[:, :, 512:576])
        nc.gpsimd.collective_compute(
            "AllGather", mybir.AluOpType.bypass,
            ins=[stg_d[:, :, :]],
            outs=[gth_d[:, :, :, :]],
            replica_groups=[list(range(NCORES))])
        nc.sync.dma_start(g_sb[:, :, :, :],
                          gth_d[:, :, :, :].rearrange("j c b t -> c j b t"))

        for h in range(NHEADS):
            pair_seg(6 + h, 1, h, SEGB)

        # ---- fill halos from gathered slabs (one-hot select chains) ----
        for b in range(B):
            nc.vector.memset(ah_ext[:, b, 576:672], 0.0)
            nc.vector.memset(ah_ext[:, b, 0:64], 0.0)
            for j in range(NCORES):
                nc.vector.scalar_tensor_tensor(
                    ah_ext[:, b, 576:672], in0=g_sb[:, j, b, 0:96],
                    scalar=selr_sb[:, j:j + 1], in1=ah_ext[:, b, 576:672],
                    op0=mybir.AluOpType.mult, op1=mybir.AluOpType.add)
                nc.vector.scalar_tensor_tensor(
                    ah_ext[:, b, 0:64], in0=g_sb[:, j, b, 96:160],
                    scalar=sell_sb[:, j:j + 1], in1=ah_ext[:, b, 0:64],
                    op0=mybir.AluOpType.mult, op1=mybir.AluOpType.add)

        # ---- scrambled projection ----
        for b in range(B):
            xtp = misc_ps.tile([128, 512], F32, tag="m")
            for j in range(16):
                nc.tensor.matmul(
                    xtp[0:96, 32 * j:32 * j + 32],
                    lhsT=ah_ext[:, b, 64 + 32 * j: 64 + 32 * j + 96],
                    rhs=s3_sb[:, j % 3, :], start=True, stop=True)
            # wrap groups (rows crossing a plane boundary; kept only on core 0)
            xtw = misc_ps.tile([128, 512], F32, tag="m")
            for w in range(2):
                sa = small_p.tile([96, 96], BF16, tag="sa")
                sb_ = small_p.tile([96, 96], BF16, tag="sb")
                nc.vector.memset(sa[:, :], 0.0)
                nc.vector.memset(sb_[:, :], 0.0)
                nc.vector.tensor_copy(sa[:, 0:64 - 32 * w],
                                      ah_ext[:, b, 32 * w: 64])
                nc.vector.tensor_copy(sb_[:, 64 - 32 * w:96],
                                      ah_ext[:, b, 64: 96 + 32 * w])
                nc.tensor.matmul(xtw[0:96, 32 * w:32 * w + 32], lhsT=sa[:, :],
                                 rhs=s3_sb[:, w, :], start=True, stop=False)
                nc.tensor.matmul(xtw[0:96, 32 * w:32 * w + 32], lhsT=sb_[:, :],
                                 rhs=sp2_sb[:, w, :], start=False, stop=True)
            nc.vector.tensor_copy(xt_sb[:, b, 0:512], xtp[0:96, :])
            nc.vector.tensor_copy(xt_sb[:, b, 512:576], xtw[0:96, 0:64])

        for b in range(B):
            yps = misc_ps.tile([128, 512], F32, tag="m")
            nc.tensor.matmul(yps[0:96, :], lhsT=pw_sb[:, :], rhs=xt_sb[:, b, 0:512],
                             start=True, stop=True)
            nc.vector.tensor_scalar_add(ysb[:, b, 0:512], yps[0:96, :], pb_sb[:, :])
            ypw = misc_ps.tile([128, 512], F32, tag="m")
            nc.tensor.matmul(ypw[0:96, 0:64], lhsT=pw_sb[:, :],
                             rhs=xt_sb[:, b, 512:576], start=True, stop=True)
            nc.vector.tensor_scalar_add(ysb[:, b, 512:576], ypw[0:96, 0:64],
                                        pb_sb[:, :])
            nc.sync.dma_start(y_d[b], ysb[:, b, :])

    nc.compile()
    return nc


_PROG = None


def _prep_inputs(x, qkv_w, qkv_b, proj_w, proj_b):
    x = np.asarray(x, np.float32)
    qkv_w = np.asarray(qkv_w, np.float32)
    qkv_b = np.asarray(qkv_b, np.float32)
    proj_w = np.asarray(proj_w, np.float32)
    proj_b = np.asarray(proj_b, np.float32)

    xt = x.transpose(0, 2, 1).reshape(B, C, H, W)
    xpad = np.zeros((B, C, H + 2, WP), np.float32)
    xpad[:, :, 1:H + 1, 1:W + 1] = xt

    xkv = np.zeros((B, 64, H + 4, WP), np.float32)
    xkv[:, :, 0:H + 2, :] = xpad[:, 32:96]

    xqs = []
    for i in range(NCORES):
        buf = np.zeros((B, 32, QROWS + 4, WP), np.float32)
        buf[:, :, 0:QROWS + 2, :] = xpad[:, 0:32, i * QROWS: i * QROWS + QROWS + 2, :]
        xqs.append(buf)

    w = qkv_w.reshape(3 * C, 3, 3)
    wm = np.zeros((3, 3, 97, 96), np.float32)  # [g, dx, k=(dy*32+c), o]
    o = np.arange(96)
    for g in range(3):
        for dy in range(3):
            for dx in range(3):
                wm[g, dx, dy * 32 + o // 3, o] = w[g * 96 + o, dy, dx]
        wm[g, 0, 96, :] = qkv_b[g * 96:(g + 1) * 96]

    import ml_dtypes
    bf16 = ml_dtypes.bfloat16
    xqs = [a.astype(bf16) for a in xqs]
    xkv = xkv.astype(bf16)
    wm = wm.astype(bf16)
    pw = np.ascontiguousarray(proj_w.T).astype(bf16)
    pb = np.ascontiguousarray(proj_b.reshape(96, 1))
    ones = np.ones((1, B * LK), bf16)

    # per-core one-hot selection matrices + halo-select vectors
    s3s, sp2s, selrs, sells = [], [], [], []
    for i in range(NCORES):
        s3 = np.zeros((3, 96, 32), np.float32)
        for a in range(3):
            r = (i + a) % 3
            for s in range(32):
                s3[a, 3 * s + r, s] = 1.0
        sp2 = np.zeros((2, 96, 32), np.float32)
        for wdx in range(2):
            for s in range(32):
                sp2[wdx, 3 * s + wdx + 1, s] = 1.0
        selr = np.zeros((96, 8), np.float32)
        selr[:, (i + 1) % 8] = 1.0
        sell = np.zeros((96, 8), np.float32)
        sell[:, (i - 1) % 8] = 1.0
        s3s.append(s3.astype(bf16))
        sp2s.append(sp2.astype(bf16))
        selrs.append(selr)
        sells.append(sell)
    return xqs, xkv, wm, pw, pb, ones, s3s, sp2s, selrs, sells


def _in_maps(inputs):
    xqs, xkv, wm, pw, pb, ones, s3s, sp2s, selrs, sells = _prep_inputs(
        inputs["x"], inputs["qkv_w"], inputs["qkv_b"],
        inputs["proj_w"], inputs["proj_b"])
    return [
        {"xq": xqs[i], "xkv": xkv, "wm": wm, "pw": pw, "pb": pb, "ones": ones,
         "s3": s3s[i], "sp2": sp2s[i], "selr": selrs[i], "sell": sells[i]}
        for i in range(NCORES)
    ]


def _col_to_n():
    """Per core: list of (column in y[b,:,0:576], output row n)."""
    maps = []
    for i in range(NCORES):
        m = []
        for j in range(16):
            if i == 7 and j >= 14:
                continue
            r = (i + j) % 3
            for s in range(32):
                n = (4096 * (3 * s + r) + 512 * i + 32 * j) // 96
                m.append((32 * j + s, n))
        if i == 0:
            for wdx in range(2):
                for s in range(32):
                    m.append((512 + 32 * wdx + s, 128 * s + 43 * wdx + 42))
        maps.append(m)
    return maps


_COLMAPS = _col_to_n()


def assemble(parts):
    """parts[i]: core i's y [B, 96, 576] -> full [B, 4096, 96]."""
    out = np.empty((B, N, 96), np.float32)
    for i, part in enumerate(parts):
        cm = _COLMAPS[i]
        cols = np.array([c for c, _ in cm])
        ns = np.array([n for _, n in cm])
        out[:, ns, :] = part[:, :, cols].transpose(0, 2, 1)
    return out


def kernel(x, qkv_w, qkv_b, proj_w, proj_b, H=64, W=64):
    global _PROG
    if _PROG is None:
        _PROG = _build_program()
    nc = _PROG

    in_maps = _in_maps({"x": x, "qkv_w": qkv_w, "qkv_b": qkv_b,
                        "proj_w": proj_w, "proj_b": proj_b})
    res = run_bass_kernel_spmd(nc, in_maps, list(range(NCORES)))
    parts = [np.asarray(res.results[i]["y"]) for i in range(NCORES)]
    return assemble(parts)
